# revision 62
# baseline (speedup 1.0000x reference)
"""Multi-head attention (B=2, T=4096, D=768, H=12) as a Bass/Tile kernel
for 8 Trainium2 NeuronCores.

Sharding: cores 0-3 own batch 0, cores 4-7 own batch 1; each core owns 3
heads (A, B, C). Each core computes x@Wq/Wk/Wv for its heads, attention,
and its heads' partial O-projection; the host sums the 4 per-batch
partials. b_o and the entire effect of b_v ride the normalized aug row
(row 64 of wo2).

Score scaling: W_q/b_q and W_k/b_k are host-prescaled by 1/sqrt(128) so
raw score matmuls produce w = s/128. exp(s/8) = exp(16w) is then either
ACT Exp with scale=16, or ONE custom DVE op (8 ALU stages):
e = (C + w*(A + B*w))^16 via 4 squarings (quadratic minimax base fit of
e^w on |w| <= 21/128).

Per-core pipeline:
  A) Q^T/K^T [dk, T] fp8 DoubleRow-packed from matmuls with W stationary
     / x^T moving. V lands in vall [128, NKC*193] bf16: per key-chunk
     block [V_A(64)|V_B(64)|V_C(64)|ones(1)]; the ones column feeds the
     sumexp rides of the attnV accumulation.
  B) Per (q-chunk, head): scores^T[k, q] into PSUM sct [128,1024]
     (2 key chunks), exp on ACT (native Exp) or DVE (single custom op
     EXP_QUAD16_ANT), attnV accumulates into acc [q, 4*65] PSUM
     (V matmul 64 rows + ones matmul 1 row per (kc, qsub); head C's V+ones
     block is contiguous -> single 65-row matmul).
  C) Drain: rc = 1/sumexp; ONE broadcast tensor_tensor normalizes
     acc -> asb (bf16, [128, 4*128]-padded blocks; heads A,B share asb01
     column halves). XBAR DMA-transposes (SP queue, 112ns) turn asb into
     aT01/aT2 [128, 512] directly in SBUF (no PE transpose, no copy).
     O-proj: po [128,384] halves, K=128 (aT01) + K=65 (aT2) accumulated;
     DVE evicts to SBUF, Pool SWDGE DMAs to HBM.

Engine budget per core (v1 cost model): ACT ~261us (exp 251 tiles),
DVE ~261us (exp 133 tiles + PSUM-side support), PE ~239us, SP ~57us,
Pool ~62us.
"""
import sys
import os
import numpy as np

try:
    import jax
    jax.config.update("jax_compilation_cache_dir", "/tmp/jax_cache_mha")
    jax.config.update("jax_persistent_cache_min_compile_time_secs", 1.0)
except Exception:
    pass

if "/opt/trn_rl_repo" not in sys.path:
    sys.path.insert(0, "/opt/trn_rl_repo")

N_CORES = 8
B, T, D, H, DK = 2, 4096, 768, 12, 64
HPC = 3  # heads per core
NKC = T // 128   # 32 key chunks
NQC = T // 512   # 8 query chunks
VW = HPC * (DK + 1)  # 195: per-kc vall block [V_A|1|V_B|1|V_C|1]

# e = (EXP_C + w*(EXP_A + EXP_B*w))^16 ~= exp(16*w) for |w| <= 21/128
EXP_A = 1.0033549147078078
EXP_B = 0.49915769166118285
EXP_C = 1.000022542129432
QK_SCALE = 1.0 / np.sqrt(128.0)  # applied to W_q,b_q,W_k,b_k host-side

N_DVE = int(os.environ.get("KB_ND", "144"))   # exp tiles on DVE (of 384)
NPAIR = 3  # PSUM score-ring pair-slots of [128, 1024] f32 (2 banks each)

_cache = {}


def _exp_schedule():
    """384 exp-tile modes: N_DVE 'D' tiles spread evenly among 'A'."""
    n = NQC * HPC * 16
    skip = int(os.environ.get("KB_SKIP", "0"))
    sched = ['A'] * n
    accd = 0.0
    fd = N_DVE / max(1, n - skip)
    for g in range(skip, n):
        accd += fd
        if accd >= 1.0:
            sched[g] = 'D'
            accd -= 1.0
    return sched


def _register_exp_ops():
    """Register the single-op DVE exp (idempotent)."""
    import concourse.dve_ops as dve_ops
    from concourse.dve_ops import DveOp, DveOpSpec
    from concourse.dve_spec import Spec, Src0, C0, C1, C2, One, lower as _lower

    def reg(name, spec):
        if name in dve_ops.CUSTOM_DVE_SPECS:
            return next(op for op in dve_ops.OPS if op.name == name)
        sha = {}
        for ver in ("v3", "v4"):
            sha[ver] = DveOpSpec(name=name, opcode=0,
                                 uops=_lower(spec, ver=ver)).sha(ver)
        op = DveOp(name, spec, subdim=False, uops_sha=sha)
        dve_ops.OPS.append(op)
        dve_ops.CUSTOM_DVE_SPECS[name] = spec
        dve_ops._SUB_OPCODE_FOR_NAME[name] = (
            max(dve_ops._SUB_OPCODE_FOR_NAME.values()) + 1)
        assert dve_ops._SUB_OPCODE_FOR_NAME[name] < 0x20
        return op

    u = C2 + Src0 * (C0 + Src0 * C1)
    u2 = u * u
    u4 = u2 * u2
    u8 = u4 * u4
    exp_q16 = reg("EXP_QUAD16_ANT", Spec(
        body=u8 * u8,
        reference=lambda in0, in1, c0, c1, c2: (
            (lambda w: (c2 + w * (c0 + w * c1)) ** 16)(
                in0.astype(np.float32))),
    ))
    return exp_q16


def _build_nc():
    import concourse.bass as bass  # noqa: F401
    import concourse.mybir as mybir
    import concourse.tile as tile
    from concourse import bacc

    f32 = mybir.dt.float32
    f32r = mybir.dt.float32r
    bf16 = mybir.dt.bfloat16
    fp8 = mybir.dt.float8e4
    AF = mybir.ActivationFunctionType
    PM = mybir.MatmulPerfMode
    ALU = mybir.AluOpType

    EXP_Q16 = _register_exp_ops()
    SCHED = _exp_schedule()
    # e (softmax weights) dtype: fp8e4m3 halves SBUF per tile; the 3-4%
    # per-weight quantization error averages out over the 4096-key sum.
    EDT = (mybir.dt.float8e4 if int(os.environ.get("KB_E8", "1"))
           else mybir.dt.bfloat16)

    NOXBAR = int(os.environ.get("KB_NOXBAR", "0"))

    nc = bacc.Bacc(None, target_bir_lowering=False)
    xbT = nc.dram_tensor("xbT", [D, T], f32r, kind="ExternalInput")
    identb_d = nc.dram_tensor("identb", [128, 128], bf16,
                              kind="ExternalInput")
    # wqk: 3 stationary groups of 128 cols: [Q_A|Q_B], [K_A|K_B], [Q_C|K_C]
    wqk = nc.dram_tensor("wqk", [D, 384], f32r, kind="ExternalInput")
    wvb = nc.dram_tensor("wvb", [D, 256], f32r, kind="ExternalInput")
    bpack = nc.dram_tensor("bpack", [128, 3], f32, kind="ExternalInput")
    wo01_d = nc.dram_tensor("wo01", [128, D], bf16, kind="ExternalInput")
    wo2_d = nc.dram_tensor("wo2", [65, D], bf16, kind="ExternalInput")
    o = nc.dram_tensor("o", [T, D], f32, kind="ExternalOutput")

    with tile.TileContext(nc) as tc:
        with tc.tile_pool(name="pers", bufs=1) as pers, \
             tc.tile_pool(name="expp", bufs=88) as expp, \
             tc.tile_pool(name="sbc", bufs=2) as sbc, \
             tc.tile_pool(name="asbp", bufs=2) as asbp, \
             tc.tile_pool(name="aTp", bufs=2) as aTp, \
             tc.tile_pool(name="outp", bufs=2) as outp, \
             tc.tile_pool(name="psB", bufs=1, space="PSUM") as psB:
            # persistents load on the Pool SWDGE queue: SP is reserved for
            # the phase-A xt stream so the first projection starts early
            bias_t = pers.tile([128, 3], f32, tag="bias")
            nc.gpsimd.dma_start(out=bias_t, in_=bpack[:, :])
            wo01 = pers.tile([128, D], bf16, tag="wo01")
            wo2 = pers.tile([65, D], bf16, tag="wo2")
            identb = pers.tile([128, 128], bf16, tag="identb")

            # Q/K per head in fp8e4m3 DoubleRow-packed layout [32, 2, T]
            q8 = [pers.tile([32, 2 * T], fp8, tag=f"q8_{h}", name=f"q8_{h}")
                  for h in range(HPC)]
            k8 = [pers.tile([32, 2 * T], fp8, tag=f"k8_{h}", name=f"k8_{h}")
                  for h in range(HPC)]
            q8v = [t.rearrange("p (i t) -> p i t", i=2) for t in q8]
            k8v = [t.rearrange("p (i t) -> p i t", i=2) for t in k8]
            # V per key chunk: per-head contiguous [V_h(64)|ones(1)] blocks
            vall = pers.tile([128, NKC * VW], bf16, tag="vall", name="vall")
            # init only the 96 ones-columns (col 64 of each 65-block)
            nc.vector.memset(vall.rearrange(
                "p (b c) -> p b c", c=DK + 1)[:, :, 64:65], 1.0)

            # ---- PSUM score ring: NPAIR slots of [128, 1024] f32 ---------
            # Triple-buffered score tiles (pool tag "sct"); phase-A pj/vp
            # and o-proj PSUM allocate from the same tag (allocation order
            # gives write-after-read slack of ~2.5 steps, hiding the
            # exp->scores turnaround that a 2-buffer scheme serializes on).
            def ring_next():
                return psB.tile([128, 1024], f32, tag="sct", bufs=NPAIR,
                                name="sct")

            # ---- per-unit machinery -------------------------------------
            gctr = [0]  # global exp tile counter

            def begin_unit(qc, h):
                st = {"qc": qc, "h": h}
                alloc_acc(st)
                return st

            # KB_MS=1: skip the acc memset; the unit's first attnV matmul
            # uses start=True, which zeroes the whole 2KB PSUM bank the acc
            # occupies (all later matmuls of the unit accumulate after it
            # in PE program order).
            MS = int(os.environ.get("KB_MS", "1"))

            def alloc_acc(st):
                acc = psB.tile([128, 4 * 65], f32, tag="acc", bufs=2,
                               name="acc")
                if not MS:
                    nc.vector.memset(acc, 0.0)
                st["acc"] = acc
                st["first"] = bool(MS)

            def emit_scores(st, ss):
                h = st["h"]
                qs = slice(st["qc"] * 512, (st["qc"] + 1) * 512)
                sct = ring_next()
                for j, kc in ((0, 2 * ss), (1, 2 * ss + 1)):
                    nc.tensor.matmul(
                        sct[:, j * 512:(j + 1) * 512],
                        k8v[h][:, :, kc * 128:(kc + 1) * 128],
                        q8v[h][:, :, qs],
                        start=True, stop=True, perf_mode=PM.DoubleRow,
                        skip_group_check=True)
                return sct

            def emit_exp(sct):
                mode = SCHED[gctr[0]]
                gctr[0] += 1
                e = expp.tile([128, 1024], EDT, tag="e", name="e")
                if mode == 'A':
                    nc.scalar.activation(e, sct, AF.Exp, scale=16.0)
                else:
                    nc.vector._custom_dve(EXP_Q16, out=e, in0=sct,
                                          s0=EXP_A, s1=EXP_B, imm2=EXP_C)
                return e

            def emit_attnv(st, ss, e):
                h, acc = st["h"], st["acc"]
                for j2 in range(8):
                    kc = 2 * ss + j2 // 4
                    qsub = j2 % 4
                    esl = e[:, (j2 // 4) * 512 + qsub * 128:
                            (j2 // 4) * 512 + (qsub + 1) * 128]
                    last = (ss == 15 and j2 >= 4)
                    first = st.pop("first", False) and j2 == 0
                    nc.tensor.matmul(
                        acc[:, qsub * 65:(qsub + 1) * 65],
                        esl, vall[:, kc * VW + h * 65:kc * VW + h * 65 + 65],
                        start=first, stop=last, skip_group_check=True)

            # ---- drain of a finished unit --------------------------------
            asb_store = {}

            def drain_norm(st):
                """rc = 1/sumexp; one broadcast TT normalizes acc into the
                padded asb blocks (h0: cols 0:64, h1: 64:128 of asb01;
                h2: 0:65 of asb2)."""
                h, qc, acc = st["h"], st["qc"], st["acc"]
                rc4 = sbc.tile([128, 4], f32, tag="rc", name="rc4")
                accv = acc.rearrange("p (q c) -> p q c", c=65)
                nc.vector.reciprocal(rc4, accv[:, :, 64])
                if h == 0:
                    asb01 = asbp.tile([128, 512], bf16, tag="asb01",
                                      name="asb01")
                    asb_store[(qc, "01")] = asb01
                if h < 2:
                    asb01 = asb_store[(qc, "01")]
                    dst = asb01.rearrange("p (q c) -> p q c", c=128)[
                        :, :, h * 64:(h + 1) * 64]
                    src = accv[:, :, 0:64]
                    ncols = 64
                else:
                    asb2 = asbp.tile([128, 512], bf16, tag="asb2",
                                     name="asb2")
                    # init the pad columns the XBAR transpose reads
                    nc.gpsimd.memset(asb2, 0.0)
                    asb_store[(qc, "2")] = asb2
                    dst = asb2.rearrange("p (q c) -> p q c", c=128)[
                        :, :, 0:65]
                    src = accv
                    ncols = 65
                if int(os.environ.get("KB_NOBCAST", "0")):
                    for qsub in range(4):
                        nc.vector.tensor_scalar_mul(
                            dst[:, qsub, :], src[:, qsub, :],
                            rc4[:, qsub:qsub + 1])
                else:
                    rcb = rc4.unsqueeze(2).broadcast_to([128, 4, ncols])
                    nc.vector.tensor_tensor(dst, src, rcb, ALU.mult)

            aT_store = {}

            def drain_transpose(st):
                """Transpose asb -> aT [128, 512] via PE identity-transpose
                through a ring slot + one DVE copy. h1: joint h0|h1
                [128,128] blocks -> aT01; h2: -> aT2. (The XBAR
                DMA-transpose mis-executes on HW, so PE does it.)"""
                h, qc = st["h"], st["qc"]
                if h == 0:
                    return
                key = "01" if h == 1 else "2"
                asb = asb_store[(qc, key)]
                aT = aTp.tile([128, 512], bf16, tag=f"aT{key}",
                              name=f"aT{key}")
                aT_store[(qc, key)] = aT
                # stage the transpose in the acc bank just freed by this
                # drain's norm read (acc tag cycles A,B; the PE transpose
                # waits only on that read)
                tp = psB.tile([128, 4 * 65], f32, tag="acc", bufs=2,
                              name="tp").bitcast(bf16)[:, 0:512]
                for qsub in range(4):
                    nc.tensor.matmul(
                        tp[:, qsub * 128:(qsub + 1) * 128],
                        asb[:, qsub * 128:(qsub + 1) * 128],
                        identb, is_transpose=True,
                        skip_group_check=True)
                nc.vector.tensor_copy(aT, tp)

            ot_store = {}

            def emit_oproj(qc, i, half, aT01, aT2):
                """One 384-col half of one 128-row o-proj chunk (a single
                ring slot per call, so consecutive services spread the
                ring pressure)."""
                if half == 0:
                    ot_store[(qc, i)] = outp.tile([128, D], f32, tag="ot",
                                                  name="ot")
                ot = ot_store[(qc, i)]
                isl = slice(i * 128, (i + 1) * 128)
                cs = slice(half * 384, (half + 1) * 384)
                po = ring_next()[:, 0:384]
                nc.tensor.matmul(po, aT01[:, isl], wo01[:, cs],
                                 start=True, stop=False,
                                 skip_group_check=True)
                nc.tensor.matmul(po, aT2[0:65, isl], wo2[:, cs],
                                 start=False, stop=True,
                                 skip_group_check=True)
                nc.vector.tensor_copy(ot[:, cs], po)
                if half == 1:
                    r0 = qc * 512 + i * 128
                    nc.gpsimd.dma_start(out=o[r0:r0 + 128, :], in_=ot)
                    del ot_store[(qc, i)]

            # ============ Pipeline orchestration ==========================
            DEPTH = int(os.environ.get("KB_DEPTH", "13"))
            pend = []  # deque of (st, ss, e) awaiting attnV emission
            drain_q = []
            oproj_q = []
            cur_drain = [None]

            def retire_one():
                pst, pss, pe = pend.pop(0)
                if "acc" not in pst:
                    alloc_acc(pst)
                emit_attnv(pst, pss, pe)
                if pss == 15:
                    drain_q.append(pst)

            def unit_step(st, ss):
                # retire pending attnV BEFORE the scores matmul: scores may
                # block at the PE queue head on its PSUM slot (WAR with the
                # exp reader), and attnV behind it would stall despite
                # being ready. Retire 2 while a backlog exists.
                n = 2 if len(pend) > DEPTH + 1 else 1
                for _ in range(n):
                    if len(pend) >= DEPTH:
                        retire_one()
                sct = emit_scores(st, ss)
                e = emit_exp(sct)
                pend.append((st, ss, e))

            pend2 = []  # phase-A emissions of unit (0,2): attnV deferred

            def emit_only(st, ss):
                """Phase-A helper: scores+exp for a unit whose attnV retires
                are deferred to the main loop (no acc yet)."""
                sct = emit_scores(st, ss)
                e = emit_exp(sct)
                pend2.append((st, ss, e))

            def service():
                """One slot of drain/o-proj work, called once per step."""
                if cur_drain[0] is not None:
                    d = cur_drain[0]
                    drain_transpose(d)
                    if d["h"] == 2:
                        oproj_q.extend((d["qc"], i, half)
                                       for i in range(4) for half in range(2))
                    cur_drain[0] = None
                elif drain_q:
                    d = drain_q.pop(0)
                    drain_norm(d)
                    cur_drain[0] = d
                elif oproj_q:
                    pqc, i, half = oproj_q.pop(0)
                    emit_oproj(pqc, i, half, aT_store[(pqc, "01")],
                               aT_store[(pqc, "2")])

            def pipe_flush():
                while pend:
                    retire_one()
                    service()

            # ============ Phase A: projections + deep emission fill =======
            # u0/u1 run inline (their attnVs retire in phase A). Units
            # (0,2)..(2,2) emit scores+exp only (attnV deferred to the main
            # loop); a step of unit (qc,h) is eligible at tcb when its q8
            # block exists (tcb > qc) and its key chunks 2ss,2ss+1 came
            # from an earlier tcb (ss <= 2(tcb-1)+1). Emissions interleave
            # with the projection groups so exp work spreads across the
            # whole tcb instead of clustering at its end.
            st0 = {"qc": 0, "h": 0}
            st1 = {"qc": 0, "h": 1}
            defer = [{"qc": qc, "h": h} for (qc, h) in
                     [(0, 2), (1, 0), (1, 1), (1, 2),
                      (2, 0), (2, 1), (2, 2)]]
            defer_ss = [0] * len(defer)

            def emit_backlog(tcb, budget):
                for _ in range(budget):
                    for j, st in enumerate(defer):
                        if (tcb >= st["qc"] + 1 and defer_ss[j] < 16
                                and defer_ss[j] <= 2 * (tcb - 1) + 1):
                            emit_only(st, defer_ss[j])
                            defer_ss[j] += 1
                            break
                    else:
                        return

            with tc.tile_pool(name="pA", bufs=1) as pA, \
                 tc.tile_pool(name="stgp", bufs=3) as stgp, \
                 tc.tile_pool(name="xTp", bufs=8) as xTp:
                wqk_t = [pA.tile([128, 384], f32r, tag=f"wqk{dc}",
                                 name=f"wqk{dc}") for dc in range(6)]
                wvb_t = [pA.tile([128, 256], f32r, tag=f"wvb{dc}",
                                 name=f"wvb{dc}") for dc in range(6)]
                for dc in range(6):
                    nc.gpsimd.dma_start(out=wqk_t[dc],
                                        in_=wqk[dc * 128:(dc + 1) * 128, :])
                for dc in range(6):
                    nc.gpsimd.dma_start(out=wvb_t[dc],
                                        in_=wvb[dc * 128:(dc + 1) * 128, :])
                # late persistents after the hot weight loads
                nc.gpsimd.dma_start(out=wo01, in_=wo01_d[:, :])
                nc.gpsimd.dma_start(out=wo2, in_=wo2_d[:, :])
                nc.gpsimd.dma_start(out=identb, in_=identb_d[:, :])

                for tcb in range(NQC):
                    tcols = slice(tcb * 512, (tcb + 1) * 512)
                    xts = []
                    for dc in range(6):
                        xt = xTp.tile([128, 512], f32r, tag="xT")
                        nc.sync.dma_start(
                            out=xt, in_=xbT[dc * 128:(dc + 1) * 128, tcols])
                        xts.append(xt)
                    # g0 = [Q_A|Q_B], g1 = [K_A|K_B], g2 = [Q_C|K_C]
                    dests = ((q8v[0], q8v[1]), (k8v[0], k8v[1]),
                             (q8v[2], k8v[2]))
                    for g in range(3):
                        pj = ring_next()[:, 0:512]
                        for dc in range(6):
                            nc.tensor.matmul(
                                pj, wqk_t[dc][:, g * 128:(g + 1) * 128],
                                xts[dc], start=(dc == 0), stop=(dc == 5),
                                skip_group_check=True)
                        stg = stgp.tile([128, 512], fp8, tag="stg",
                                        name="stg")
                        nc.vector.tensor_scalar_add(stg, pj,
                                                    bias_t[:, g:g + 1])
                        da, db = dests[g]
                        nc.sync.dma_start(out=da[:, :, tcols],
                                          in_=stg[0:64, :])
                        nc.sync.dma_start(out=db[:, :, tcols],
                                          in_=stg[64:128, :])
                        emit_backlog(tcb, 2)
                    for i in range(4):
                        kc = tcb * 4 + i
                        vp = ring_next()[:, 0:256]
                        for dc in range(6):
                            nc.tensor.matmul(
                                vp, xts[dc][:, i * 128:(i + 1) * 128],
                                wvb_t[dc], start=(dc == 0), stop=(dc == 5),
                                skip_group_check=True)
                        vdst = vall[:, kc * VW:(kc + 1) * VW].rearrange(
                            "p (b c) -> p b c", c=65)[:, :, 0:64]
                        nc.vector.tensor_copy(
                            vdst, vp[:, 0:192].rearrange(
                                "p (b c) -> p b c", c=64))
                        emit_backlog(tcb, 2)
                    # u0/u1 supersteps 2t, 2t+1 (kc <= 4t+3)
                    unit_step(st0, 2 * tcb)
                    unit_step(st1, 2 * tcb)
                    unit_step(st0, 2 * tcb + 1)
                    unit_step(st1, 2 * tcb + 1)

            # ============ Main loop: remaining 21 units, pipelined ========
            # Each step retires pending attnVs (2 while the phase-A backlog
            # lasts), then emits scores+exp, then services one slot of
            # drain/transpose/o-proj work.
            pend.extend(pend2)
            units = [(qc, h) for qc in range(NQC) for h in range(HPC)]

            for idx, (qc, h) in enumerate(units[2:]):
                if idx < len(defer):
                    st, start_ss = defer[idx], defer_ss[idx]
                else:
                    st, start_ss = {"qc": qc, "h": h}, 0
                # during the final unit, drain the attnV backlog early so
                # the tail after the last emission is short
                last_unit = idx == len(units[2:]) - 1
                for ss in range(start_ss, 16):
                    if last_unit and len(pend) >= 2:
                        retire_one()
                    unit_step(st, ss)
                    service()

            pipe_flush()
            while drain_q or cur_drain[0] is not None or oproj_q:
                service()

    nc.finalize()
    return nc


def _get_nc():
    if "nc" not in _cache:
        _cache["nc"] = _build_nc()
    return _cache["nc"]


def _make_in_maps(x, W_q, b_q, W_k, b_k, W_v, b_v, W_o, b_o):
    import ml_dtypes
    bf = ml_dtypes.bfloat16
    s = np.float32(QK_SCALE)
    W_q = W_q * s
    b_q = b_q * s
    W_k = W_k * s
    b_k = b_k * s
    in_maps = []
    for c in range(N_CORES):
        b = c // 4
        h0 = (c % 4) * HPC   # first global head on this core
        c0 = h0 * DK         # first column of this core's heads
        # g0 = [Q_A | Q_B], g1 = [K_A | K_B], g2 = [Q_C | K_C]
        g0 = W_q[:, c0:c0 + 128]
        g1 = W_k[:, c0:c0 + 128]
        g2 = np.concatenate([W_q[:, c0 + 128:c0 + 192],
                             W_k[:, c0 + 128:c0 + 192]], axis=1)
        wqk = np.concatenate([g0, g1, g2], axis=1)

        bpack = np.zeros((128, 3), np.float32)
        bpack[:, 0] = b_q[c0:c0 + 128]
        bpack[:, 1] = b_k[c0:c0 + 128]
        bpack[0:64, 2] = b_q[c0 + 128:c0 + 192]
        bpack[64:128, 2] = b_k[c0 + 128:c0 + 192]

        wo01 = W_o[c0:c0 + 128, :].astype(bf)
        wo2 = np.zeros((65, D), np.float32)
        wo2[0:64, :] = W_o[c0 + 128:c0 + 192, :]
        # b_v's effect on the normalized output is the constant
        # b_v_h @ W_o_h per head; all three heads ride the aug row.
        wo2[64, :] = b_v[c0:c0 + 192] @ W_o[c0:c0 + 192, :]
        if c % 4 == 0:
            wo2[64, :] += b_o  # b_o folded once per batch

        in_maps.append({
            "xbT": np.ascontiguousarray(x[b].T),
            "identb": np.eye(128, dtype=bf),
            "wqk": np.ascontiguousarray(wqk),
            "wvb": np.ascontiguousarray(np.concatenate(
                [W_v[:, c0:c0 + 192], np.zeros((D, 64), np.float32)],
                axis=1)),
            "bpack": bpack,
            "wo01": np.ascontiguousarray(wo01),
            "wo2": wo2.astype(bf),
        })
    return in_maps


def kernel(**inputs):
    from concourse.bass_utils import run_bass_kernel_spmd

    args = {k: np.asarray(v, dtype=np.float32) for k, v in inputs.items()}
    in_maps = _make_in_maps(
        args["x"], args["W_q"], args["b_q"], args["W_k"], args["b_k"],
        args["W_v"], args["b_v"], args["W_o"], args["b_o"])

    nc = _get_nc()
    trace = bool(int(os.environ.get("KBENCH_TRACE", "0")))
    res = run_bass_kernel_spmd(nc, in_maps, core_ids=list(range(N_CORES)),
                               trace=trace)
    _cache["last_result"] = res

    out = np.zeros((B, T, D), np.float32)
    for c in range(N_CORES):
        out[c // 4] += res.results[c]["o"]
    return out
